# revision 1
# baseline (speedup 1.0000x reference)
"""Causal self-attention (B=2, T=2048, C=1024, H=16) on 8 Trainium2 cores.

Sharding: data-parallel over batch (2) x tensor-parallel over heads (4 groups
of 4 heads). Core c handles batch b = c//4, head group g = c%4 (heads 4g..4g+3).
Each core computes its qkv column slice, full causal TxT attention for its 4
heads, and a partial row-parallel projection. Host sums the 4 partial proj
outputs per batch and adds b_proj.

Device kernel layout notes:
- everything feature-major ("transposed"): qT/kT [d, t] so the PE contraction
  dims line up without any on-device transposes (host supplies x pre-transposed)
- matmuls run as float32r (FP22 mantissa truncation; full-rate streaming,
  unlike true fp32 which costs 4 cycles/row)
- softmax without max-subtraction (logits are ~N(0,1); exp is safe in fp32)
- the two heads of a pair occupy partitions 0-63 / 64-127 of the qT/kT chunk,
  so their K=64 score matmuls run concurrently in disjoint PE row quadrants
- the AV stationary operand is zero-padded to M=128 with an embedded all-ones
  column, so one matmul per head yields both the weighted values (rows 0-63 or
  64-127, matching the yT layout) and the softmax denominator row for free
- denominator rows are partition-broadcast with a K=128 all-ones matmul
  against a pre-zeroed staging tile; one full-width DVE reciprocal then covers
  both heads (walrus rejects K=1 / col-offset-64 fp32r matmuls on TRN2)
- causal masking: upper-triangle j-chunks are skipped entirely; diagonal
  chunks narrow the score/exp/AV column range and one [128,128] triangular
  multiplicative mask handles the partial strip
- norm and proj emission lag the attention loop by one step so the in-order
  PE instruction stream never stalls on DVE/ACT dependencies
"""

import os
import sys

sys.path.insert(0, "/opt/trn_rl_repo")

import numpy as np

P = 128
T = 2048
C = 1024
D = 64
HPC = 4          # heads per core
HD = HPC * D     # 256 qkv columns per core
CC = C // P      # 8 contraction chunks
TC = T // P      # 16 t-chunks of 128
IC = T // 512    # 4 i-chunks of 512

_NC = None
LAST_RESULTS = None


def _build_nc():
    import concourse.mybir as mybir
    import concourse.tile as tile
    from concourse import bacc
    from contextlib import ExitStack

    dt = mybir.dt
    f32 = dt.float32
    f32r = dt.float32r
    ALU = mybir.AluOpType
    ACTF = mybir.ActivationFunctionType

    nc = bacc.Bacc(
        "TRN2",
        target_bir_lowering=False,
        debug=False,
        enable_asserts=False,
        num_devices=8,
    )

    xT = nc.dram_tensor("xT", [C, T], f32r, kind="ExternalInput").ap()
    wq = nc.dram_tensor("wq", [C, HD], f32r, kind="ExternalInput").ap()
    wk = nc.dram_tensor("wk", [C, HD], f32r, kind="ExternalInput").ap()
    wv = nc.dram_tensor("wv", [C, HD], f32r, kind="ExternalInput").ap()
    bq = nc.dram_tensor("bq", [P, 2], f32, kind="ExternalInput").ap()
    bk = nc.dram_tensor("bk", [P, 2], f32, kind="ExternalInput").ap()
    bv = nc.dram_tensor("bv", [P, HD], f32, kind="ExternalInput").ap()
    wp = nc.dram_tensor("wp", [HD, C], f32r, kind="ExternalInput").ap()
    tri = nc.dram_tensor("tri", [P, P], f32, kind="ExternalInput").ap()
    tri2 = nc.dram_tensor("tri2", [P, 2 * P], f32, kind="ExternalInput").ap()
    onesp = nc.dram_tensor("onesp", [P, 2, P], f32r, kind="ExternalInput").ap()
    out = nc.dram_tensor("out", [T, C], f32, kind="ExternalOutput").ap()

    with tile.TileContext(nc) as tc, ExitStack() as ctx:
        persist = ctx.enter_context(tc.tile_pool(name="persist", bufs=1))
        qT_sb = persist.tile([P, 2, T], f32r, name="qT")    # [d%128, dchunk, t]
        kT_sb = persist.tile([P, 2, T], f32r, name="kT")
        v_sb = persist.tile([P, TC, 2, 2, P], f32r, name="v")  # [t%128, tchunk, hpair, hi, 128-padded d]
        yT_sb = persist.tile([P, 2, T], f32r, name="yT")
        wp_sb = persist.tile([P, 2, C], f32r, name="wps")
        tri_sb = persist.tile([P, P], f32, name="tris")
        tri2_sb = persist.tile([P, 2 * P], f32, name="tri2s")
        ones_sb = persist.tile([P, 2, P], f32r, name="ones")
        bq_sb = persist.tile([P, 2], f32, name="bqs")
        bk_sb = persist.tile([P, 2], f32, name="bks")
        bv_sb = persist.tile([P, 2, 2, D], f32, name="bvs")
        dsb = persist.tile([P, 2, 512], f32r, name="dsb")

        nc.sync.dma_start(wp_sb[:], wp.rearrange("(o p) n -> p o n", p=P))
        nc.sync.dma_start(tri_sb[:], tri)
        nc.sync.dma_start(tri2_sb[:], tri2)
        nc.sync.dma_start(bq_sb[:], bq)
        nc.sync.dma_start(bk_sb[:], bk)
        nc.sync.dma_start(bv_sb[:], bv.rearrange("p (hp hi d) -> p hp hi d", hi=2, d=D))
        nc.sync.dma_start(ones_sb[:], onesp)
        # zero the den staging tile once; each iteration only rewrites row 64 of
        # plane 0 / row 0 of plane 1, every other row must read as 0 for the
        # K=128 broadcast matmuls below
        nc.vector.tensor_scalar_mul(
            dsb[:, :, :], ones_sb[:, :, 0:1].to_broadcast([P, 2, 512]), 0.0
        )

        # ---------------- phase 1: qkv projections ----------------
        with (
            tc.tile_pool(name="ph1", bufs=1) as ph1,
            tc.tile_pool(name="ps1", bufs=4, space="PSUM") as ps1,
        ):
            xT_sb = ph1.tile([P, CC, T], f32r, name="xTs")
            wq_sb = ph1.tile([P, CC, HD], f32r, name="wqs")
            wk_sb = ph1.tile([P, CC, HD], f32r, name="wks")
            wv_sb = ph1.tile([P, CC, HD], f32r, name="wvs")
            # weights first so the first matmul group can start early; x load
            # split so several DMA queues run in parallel and the first t-half
            # (needed by the first two qT column groups) lands first
            xTr = xT.rearrange("(o p) t -> p o t", p=P)
            wqr = wq.rearrange("(o p) n -> p o n", p=P)
            nc.sync.dma_start(wq_sb[:, :, 0:P], wqr[:, :, 0:P])
            nc.sync.dma_start(wq_sb[:, :, P:HD], wqr[:, :, P:HD])
            for cc in range(CC):
                eng = nc.sync if cc % 2 == 0 else nc.gpsimd
                eng.dma_start(xT_sb[:, cc, 0:T // 2], xTr[:, cc, 0:T // 2])
            nc.gpsimd.dma_start(wk_sb[:], wk.rearrange("(o p) n -> p o n", p=P))
            nc.sync.dma_start(wv_sb[:], wv.rearrange("(o p) n -> p o n", p=P))
            for cc in range(CC):
                eng = nc.sync if cc % 2 == 0 else nc.gpsimd
                eng.dma_start(xT_sb[:, cc, T // 2:T], xTr[:, cc, T // 2:T])

            # qT / kT: [cout, t] = W.T @ x.T
            for W_s, B_s, dest in ((wq_sb, bq_sb, qT_sb), (wk_sb, bk_sb, kT_sb)):
                for co in range(2):
                    for tsl in range(4):
                        ps = ps1.tile([P, 512], f32, tag="qk")
                        for cc in range(CC):
                            nc.tensor.matmul(
                                ps[:],
                                W_s[:, cc, co * P:(co + 1) * P],
                                xT_sb[:, cc, tsl * 512:(tsl + 1) * 512],
                                start=(cc == 0),
                                stop=(cc == CC - 1),
                            )
                        nc.vector.tensor_tensor(
                            dest[:, co, tsl * 512:(tsl + 1) * 512],
                            ps[:],
                            B_s[:, co:co + 1].to_broadcast([P, 512]),
                            ALU.add,
                        )
            # v: natural [t, d] layout
            for tj in range(TC):
                ps = ps1.tile([P, HD], f32, tag="v")
                for cc in range(CC):
                    nc.tensor.matmul(
                        ps[:],
                        xT_sb[:, cc, tj * P:(tj + 1) * P],
                        wv_sb[:, cc, :],
                        start=(cc == 0),
                        stop=(cc == CC - 1),
                    )
                psv = ps[:].rearrange("p (hp hi d) -> p hp hi d", hi=2, d=D)
                # hi=0 weights: [v | 1 | 0...]; hi=1 weights: [1 | 0... | v].
                # The ones column makes the AV matmul also emit the softmax
                # denominator (row 64 for hi=0, row 0 for hi=1) for free.
                nc.vector.tensor_tensor(
                    v_sb[:, tj, :, 0, 0:D], psv[:, :, 0, :], bv_sb[:, :, 0, :], ALU.add
                )
                nc.vector.tensor_tensor(
                    v_sb[:, tj, :, 1, D:P], psv[:, :, 1, :], bv_sb[:, :, 1, :], ALU.add
                )
                # constant regions (memset cannot write f32r; mult/add by imm can)
                nc.vector.tensor_scalar(
                    v_sb[:, tj, :, 0, D:D + 1], psv[:, :, 0, 0:1], 0.0, 1.0,
                    ALU.mult, ALU.add,
                )
                nc.vector.tensor_scalar(
                    v_sb[:, tj, :, 1, 0:1], psv[:, :, 1, 0:1], 0.0, 1.0,
                    ALU.mult, ALU.add,
                )
                nc.vector.tensor_scalar_mul(
                    v_sb[:, tj, :, 0, D + 1:P], psv[:, :, 0, 0:D - 1], 0.0
                )
                nc.vector.tensor_scalar_mul(
                    v_sb[:, tj, :, 1, 1:D], psv[:, :, 1, 0:D - 1], 0.0
                )

        # ---------------- phase 2: attention + interleaved proj ----------------
        with (
            tc.tile_pool(name="ph2", bufs=3) as ph2,
            tc.tile_pool(name="ph3", bufs=3) as ph3,
            tc.tile_pool(name="ps2s", bufs=2, space="PSUM") as ps2s,
            tc.tile_pool(name="ps2a", bufs=2, space="PSUM") as ps2a,
        ):
            def emit_proj(cip):
                for tj in range(4 * cip, 4 * cip + 4):
                    ot = ph3.tile([P, C], f32, tag="ot")
                    pps = ps2s.tile([P, 2, 512], f32, tag="s")
                    for co in range(2):
                        for dc in range(2):
                            nc.tensor.matmul(
                                pps[:, co, :],
                                yT_sb[:, dc, tj * P:(tj + 1) * P],
                                wp_sb[:, dc, co * 512:(co + 1) * 512],
                                start=(dc == 0),
                                stop=(dc == 1),
                            )
                        nc.vector.tensor_copy(ot[:, co * 512:(co + 1) * 512], pps[:, co, :])
                    nc.gpsimd.dma_start(out[tj * P:(tj + 1) * P, :], ot[:])

            def emit_norm(hp, i0, av0, av1):
                # copy each head's denominator row into the pre-zeroed staging
                # tile, broadcast over partitions with a K=128 all-ones matmul,
                # stage through SBUF (frees the PSUM slot fast), reciprocal both
                # 64-row halves at once, scale into yT
                nc.vector.tensor_copy(dsb[D:D + 1, 0, :], av0[D:D + 1, :])
                nc.vector.tensor_copy(dsb[0:1, 1, :], av1[0:1, :])
                bps = ps2s.tile([P, 2, 512], f32, tag="s")
                nc.tensor.matmul(
                    bps[:, 0, :], ones_sb[:, 0, :], dsb[:, 1, :],
                    start=True, stop=True, skip_group_check=True,
                )
                nc.tensor.matmul(
                    bps[0:D, 0, :], ones_sb[:, 0, 0:D], dsb[:, 0, :],
                    start=True, stop=True, skip_group_check=True,
                )
                bsb = ph2.tile([P, 512], f32, tag="bsb")
                nc.scalar.copy(bsb[:, :], bps[:, 0, :])
                rec = ph2.tile([P, 512], f32, tag="rec")
                nc.vector.reciprocal(rec[:, :], bsb[:, :])
                nc.vector.tensor_tensor(
                    yT_sb[0:D, hp, i0:i0 + 512], av0[0:D, :], rec[0:D, :], ALU.mult
                )
                nc.vector.tensor_tensor(
                    yT_sb[D:P, hp, i0:i0 + 512], av1[D:P, :], rec[D:P, :], ALU.mult
                )

            pending = []
            for ci in range(IC):
                i0 = ci * 512
                njc = 4 * (ci + 1)
                for hp in range(2):
                    av0 = ps2a.tile([P, 512], f32, tag="av0")
                    av1 = ps2a.tile([P, 512], f32, tag="av1")

                    def emit_s(jc):
                        diag = jc >= 4 * ci
                        o = (jc - 4 * ci) if diag else 0
                        c0 = 2 * P if diag and o == 3 else o * P
                        sps = ps2s.tile([P, 2, 512], f32, tag="s")
                        for hi in range(2):
                            bp = D * hi
                            nc.tensor.matmul(
                                sps[:, hi, c0:512],
                                kT_sb[bp:bp + D, hp, jc * P:(jc + 1) * P],
                                qT_sb[bp:bp + D, hp, i0 + c0:i0 + 512],
                                start=True,
                                stop=True,
                                skip_group_check=True,
                            )
                        ex = ph2.tile([P, 2, 512], f32r, tag="ex")
                        nc.scalar.activation(
                            ex[:, :, c0:512],
                            sps[:, :, c0:512],
                            ACTF.Exp,
                            scale=float(D) ** -0.5,
                        )
                        if diag and o == 3:
                            # cols 256-383 are fully masked, 384-511 triangular
                            nc.vector.tensor_tensor(
                                ex[:, :, c0:512],
                                ex[:, :, c0:512],
                                tri2_sb[:, None, :].to_broadcast([P, 2, 2 * P]),
                                ALU.mult,
                            )
                        elif diag:
                            nc.vector.tensor_tensor(
                                ex[:, :, c0:c0 + P],
                                ex[:, :, c0:c0 + P],
                                tri_sb[:, None, :].to_broadcast([P, 2, P]),
                                ALU.mult,
                            )
                        return ex, c0

                    def emit_av(jc, ex, c0):
                        for hi, av in ((0, av0), (1, av1)):
                            nc.tensor.matmul(
                                av[:, c0:512],
                                v_sb[:, jc, hp, hi, :],
                                ex[:, hi, c0:512],
                                start=(jc == 0),
                                stop=(jc == njc - 1),
                                skip_group_check=True,
                            )

                    for jc in range(njc):
                        emit_av(jc, *emit_s(jc))
                    # norms lag one head-pair so the in-order PE stream never
                    # waits on the DVE den-row copies
                    pending.append((hp, i0, av0, av1))
                    if len(pending) > 1:
                        emit_norm(*pending.pop(0))
                if ci >= 1:
                    emit_proj(ci - 1)
            while pending:
                emit_norm(*pending.pop(0))
            emit_proj(IC - 1)
    nc.compile()
    return nc


def _get_nc():
    global _NC
    if _NC is None:
        _NC = _build_nc()
    return _NC


def kernel(x, W_qkv, b_qkv, W_proj, b_proj):
    global LAST_RESULTS
    from concourse import bass_utils

    x = np.asarray(x, dtype=np.float32)
    W_qkv = np.asarray(W_qkv, dtype=np.float32)
    b_qkv = np.asarray(b_qkv, dtype=np.float32)
    W_proj = np.asarray(W_proj, dtype=np.float32)
    b_proj = np.asarray(b_proj, dtype=np.float32)

    nc = _get_nc()
    tri = np.ascontiguousarray(np.triu(np.ones((P, P), dtype=np.float32)))
    tri2 = np.ascontiguousarray(
        np.concatenate([np.zeros((P, P), np.float32), tri], axis=1)
    )
    onesp = np.zeros((P, 2, P), dtype=np.float32)
    onesp[:, 0, :] = 1.0
    in_maps = []
    for c in range(8):
        b, g = divmod(c, 4)
        s = slice(HD * g, HD * g + HD)
        in_maps.append({
            "xT": np.ascontiguousarray(x[b].T),
            "wq": np.ascontiguousarray(W_qkv[:, s]),
            "wk": np.ascontiguousarray(W_qkv[:, C + HD * g:C + HD * g + HD]),
            "wv": np.ascontiguousarray(W_qkv[:, 2 * C + HD * g:2 * C + HD * g + HD]),
            "bq": np.ascontiguousarray(b_qkv[s].reshape(2, P).T),
            "bk": np.ascontiguousarray(b_qkv[C + HD * g:C + HD * g + HD].reshape(2, P).T),
            "bv": np.ascontiguousarray(
                np.broadcast_to(b_qkv[2 * C + HD * g:2 * C + HD * g + HD], (P, HD))
            ),
            "wp": np.ascontiguousarray(W_proj[s, :]),
            "tri": tri,
            "tri2": tri2,
            "onesp": onesp,
        })

    res = bass_utils.run_bass_kernel_spmd(nc, in_maps, core_ids=list(range(8)))
    LAST_RESULTS = res
    ys = []
    for b in range(2):
        y = res.results[4 * b]["out"].astype(np.float64)
        for g in range(1, 4):
            y = y + res.results[4 * b + g]["out"]
        ys.append((y + b_proj).astype(np.float32))
    return np.stack(ys, axis=0)



# revision 5
# speedup vs baseline: 1.8336x; 1.8336x over previous
"""Causal self-attention (B=2, T=2048, C=1024, H=16) on 8 Trainium2 cores.

Sharding: data-parallel over batch (2) x tensor-parallel over heads (4 groups
of 4 heads). Core c handles batch b = c//4, head group g = c%4 (heads 4g..4g+3).
Each core computes its qkv column slice, full causal TxT attention for its 4
heads, and a partial row-parallel projection. Host sums the 4 partial proj
outputs per batch and adds b_proj.

v2 design notes (vs the f32r v1):
- everything bf16 end-to-end (inputs, SBUF intermediates, output); PSUM stays
  fp32. Halves DMA traffic (6.5MB in / 4MB out per core) and enables the PE's
  fast-weight-load path (2x LDWEIGHTS for non-fp32 128-col stationaries).
- the scalar engine's exp is the true serializer (~10.5M score exps/core at
  1 elem/lane/cycle): the qkv projection phase is fused INTO the attention
  loop (qkv for t-slice tsl emits interleaved with attention for i-block
  tsl-1) so ACT starts exp'ing ~10us in and stays saturated while the PE
  retires qkv/score/AV/proj matmuls underneath it.
- AV matmuls are column-tiled pairs (head hi=0 -> PE cols 0:64, hi=1 ->
  64:128) running concurrently, each M=64 with no zero padding; the two
  score matmuls of a head pair stay row-tiled (K=64 quadrants) as in v1.
- softmax denominators: DVE accumulates masked exp tiles into two bf16
  chains (even/odd j-chunks); a column-tiled pair of K=128 ones-stationary
  matmuls then both reduces over j and broadcasts den across partitions in
  PSUM rows matching the AV layout; gpsimd drains den to SBUF and one DVE
  divide normalizes into yT. (No ACT Reciprocal: it lives in a different
  ACT table set than Exp and each switch costs ~2.7us.)
- PSUM budget exactly 8 banks: scores [128,2,512]x2 (4) + AV [128,512]x2 (2)
  + one shared round-robin tag for qkv/den/proj psum [128,512]x2 (2).
- proj output copies + DMA and den drains ride the otherwise-idle gpsimd.
"""

import os
import sys

sys.path.insert(0, "/opt/trn_rl_repo")

import numpy as np

P = 128
T = 2048
C = 1024
D = 64
HPC = 4          # heads per core
HD = HPC * D     # 256 qkv columns per core
CC = C // P      # 8 contraction chunks
TC = T // P      # 16 t-chunks of 128
IC = T // 512    # 4 i-chunks of 512

_NC = None
LAST_RESULTS = None


def _build_nc():
    import concourse.mybir as mybir
    import concourse.tile as tile
    from concourse import bacc
    from contextlib import ExitStack

    dt = mybir.dt
    f32 = dt.float32
    bf16 = dt.bfloat16
    ALU = mybir.AluOpType
    ACTF = mybir.ActivationFunctionType

    nc = bacc.Bacc(
        "TRN2",
        target_bir_lowering=False,
        debug=False,
        enable_asserts=False,
        num_devices=8,
    )

    xT = nc.dram_tensor("xT", [C, T], bf16, kind="ExternalInput").ap()
    wq = nc.dram_tensor("wq", [C, HD], bf16, kind="ExternalInput").ap()
    wk = nc.dram_tensor("wk", [C, HD], bf16, kind="ExternalInput").ap()
    wv = nc.dram_tensor("wv", [C, HD], bf16, kind="ExternalInput").ap()
    bq = nc.dram_tensor("bq", [P, 2], f32, kind="ExternalInput").ap()
    bk = nc.dram_tensor("bk", [P, 2], f32, kind="ExternalInput").ap()
    bv = nc.dram_tensor("bv", [P, HD], f32, kind="ExternalInput").ap()
    wp = nc.dram_tensor("wp", [HD, C], bf16, kind="ExternalInput").ap()
    tri = nc.dram_tensor("tri", [P, P], bf16, kind="ExternalInput").ap()
    onesd = nc.dram_tensor("onesd", [P, D], bf16, kind="ExternalInput").ap()
    out = nc.dram_tensor("out", [T, C], bf16, kind="ExternalOutput").ap()

    with tile.TileContext(nc) as tc, ExitStack() as ctx:
        persist = ctx.enter_context(tc.tile_pool(name="persist", bufs=1))
        xT_sb = persist.tile([P, CC, T], bf16, name="xTs")
        wq_sb = persist.tile([P, CC, HD], bf16, name="wqs")
        wk_sb = persist.tile([P, CC, HD], bf16, name="wks")
        wv_sb = persist.tile([P, CC, HD], bf16, name="wvs")
        wp_sb = persist.tile([P, 2, C], bf16, name="wps")
        qT_sb = persist.tile([P, 2, T], bf16, name="qT")    # [d%128, hp, t]
        kT_sb = persist.tile([P, 2, T], bf16, name="kT")
        v_sb = persist.tile([P, TC, 2, 2, D], bf16, name="v")  # [t%128, tc, hp, hi, d]
        yT_sb = persist.tile([P, 2, T], bf16, name="yT")
        tri_sb = persist.tile([P, P], bf16, name="tris")
        ones_sb = persist.tile([P, D], bf16, name="ones")
        bq_sb = persist.tile([P, 2], f32, name="bqs")
        bk_sb = persist.tile([P, 2], f32, name="bks")
        bv_sb = persist.tile([P, 2, 2, D], f32, name="bvs")

        # ---- input DMA schedule: weights + x, t-slice-major so the fused
        # qkv/attention pipeline can start as soon as slice 0 lands ----
        xTr = xT.rearrange("(o p) t -> p o t", p=P)
        wqr = wq.rearrange("(o p) n -> p o n", p=P)
        wkr = wk.rearrange("(o p) n -> p o n", p=P)
        nc.sync.dma_start(tri_sb[:], tri)
        nc.sync.dma_start(bq_sb[:], bq)
        nc.sync.dma_start(bk_sb[:], bk)
        nc.sync.dma_start(ones_sb[:], onesd)
        nc.sync.dma_start(
            bv_sb[:], bv.rearrange("p (hp hi d) -> p hp hi d", hi=2, d=D)
        )
        nc.sync.dma_start(wq_sb[:, :, 0:P], wqr[:, :, 0:P])
        nc.gpsimd.dma_start(wk_sb[:, :, 0:P], wkr[:, :, 0:P])
        for cc in range(CC):
            eng = nc.sync if cc % 2 == 0 else nc.gpsimd
            eng.dma_start(xT_sb[:, cc, 0:512], xTr[:, cc, 0:512])
        nc.sync.dma_start(wq_sb[:, :, P:HD], wqr[:, :, P:HD])
        nc.gpsimd.dma_start(wk_sb[:, :, P:HD], wkr[:, :, P:HD])
        nc.sync.dma_start(wv_sb[:], wv.rearrange("(o p) n -> p o n", p=P))
        for cc in range(CC):
            eng = nc.sync if cc % 2 == 0 else nc.gpsimd
            eng.dma_start(xT_sb[:, cc, 512:1024], xTr[:, cc, 512:1024])
        nc.gpsimd.dma_start(wp_sb[:], wp.rearrange("(o p) n -> p o n", p=P))
        for tsl in (2, 3):
            for cc in range(CC):
                eng = nc.sync if cc % 2 == 0 else nc.gpsimd
                eng.dma_start(
                    xT_sb[:, cc, tsl * 512:(tsl + 1) * 512],
                    xTr[:, cc, tsl * 512:(tsl + 1) * 512],
                )

        with (
            tc.tile_pool(name="exp", bufs=3) as exp_pool,
            tc.tile_pool(name="accp", bufs=4) as acc_pool,
            tc.tile_pool(name="denp", bufs=2) as den_pool,
            tc.tile_pool(name="otp", bufs=3) as ot_pool,
            tc.tile_pool(name="ps_s", bufs=2, space="PSUM") as ps_s,
            tc.tile_pool(name="ps_av", bufs=2, space="PSUM") as ps_av,
            tc.tile_pool(name="ps_a", bufs=2, space="PSUM") as ps_a,
        ):
            # ---------------- qkv emission closures ----------------
            def qkv_group_qk(W_s, B_s, dest, co, tsl):
                def run():
                    ps = ps_a.tile([P, 512], f32, tag="a", name="psqk")
                    for cc in range(CC):
                        nc.tensor.matmul(
                            ps[:],
                            W_s[:, cc, co * P:(co + 1) * P],
                            xT_sb[:, cc, tsl * 512:(tsl + 1) * 512],
                            start=(cc == 0),
                            stop=(cc == CC - 1),
                        )
                    nc.vector.tensor_tensor(
                        dest[:, co, tsl * 512:(tsl + 1) * 512],
                        ps[:],
                        B_s[:, co:co + 1].to_broadcast([P, 512]),
                        ALU.add,
                    )
                return run

            def qkv_group_v(tj):
                def run():
                    ps = ps_a.tile([P, 512], f32, tag="a", name="psv")
                    for cc in range(CC):
                        nc.tensor.matmul(
                            ps[:, 0:HD],
                            xT_sb[:, cc, tj * P:(tj + 1) * P],
                            wv_sb[:, cc, :],
                            start=(cc == 0),
                            stop=(cc == CC - 1),
                        )
                    psv = ps[:, 0:HD].rearrange("p (hp hi d) -> p hp hi d", hi=2, d=D)
                    nc.vector.tensor_tensor(
                        v_sb[:, tj, :, :, :], psv, bv_sb[:], ALU.add
                    )
                return run

            def qkv_closures(tsl):
                cl = []
                for co in range(2):
                    cl.append(qkv_group_qk(wq_sb, bq_sb, qT_sb, co, tsl))
                    cl.append(qkv_group_qk(wk_sb, bk_sb, kT_sb, co, tsl))
                for tj in range(4 * tsl, 4 * tsl + 4):
                    cl.append(qkv_group_v(tj))
                return cl

            # ---------------- attention emission closures ----------------
            pending = []  # lagged norms: (ci, hp, av, acc_e, acc_o)

            def emit_norm(ci, hp, av, acc_e, acc_o):
                i0 = ci * 512
                den = ps_a.tile([P, 512], f32, tag="a", name="den")
                for acc, st in ((acc_e, True), (acc_o, False)):
                    for hi in range(2):
                        nc.tensor.matmul(
                            den[hi * D:(hi + 1) * D, :],
                            ones_sb[:],
                            acc[:, hi, :],
                            start=st,
                            stop=not st,
                            skip_group_check=True,
                        )
                rec = den_pool.tile([P, 512], f32, tag="den", name="rec")
                nc.vector.reciprocal_approx_fast(out=rec[:], in_=den[:])
                nc.vector.tensor_tensor(
                    yT_sb[:, hp, i0:i0 + 512], av[:], rec[:], ALU.mult
                )

            def proj_tile(tj):
                def run():
                    ot = ot_pool.tile([P, C], bf16, tag="ot", name="ot")
                    for co in range(2):
                        pps = ps_a.tile([P, 512], f32, tag="a", name="ppsp")
                        for dc in range(2):
                            nc.tensor.matmul(
                                pps[:],
                                yT_sb[:, dc, tj * P:(tj + 1) * P],
                                wp_sb[:, dc, co * 512:(co + 1) * 512],
                                start=(dc == 0),
                                stop=(dc == 1),
                            )
                        nc.vector.tensor_copy(ot[:, co * 512:(co + 1) * 512], pps[:])
                    nc.gpsimd.dma_start(out[tj * P:(tj + 1) * P, :], ot[:])
                return run

            def attn_closures(ci):
                i0 = ci * 512
                njc = 4 * (ci + 1)
                cl = []
                for hp in range(2):
                    av = ps_av.tile([P, 512], f32, tag="av", name="av")
                    acc_e = acc_pool.tile([P, 2, 512], bf16, tag="acc", name="acce")
                    acc_o = acc_pool.tile([P, 2, 512], bf16, tag="acc", name="acco")
                    exs = {}

                    def s_step(jc, hp=hp, exs=exs):
                        def run():
                            diag = jc >= 4 * ci
                            o = (jc - 4 * ci) if diag else 0
                            c0 = o * P
                            sps = ps_s.tile([P, 2, 512], f32, tag="s", name="sps")
                            for hi in range(2):
                                bp = D * hi
                                nc.tensor.matmul(
                                    sps[:, hi, c0:512],
                                    kT_sb[bp:bp + D, hp, jc * P:(jc + 1) * P],
                                    qT_sb[bp:bp + D, hp, i0 + c0:i0 + 512],
                                    start=True,
                                    stop=True,
                                    skip_group_check=True,
                                )
                            ex = exp_pool.tile([P, 2, 512], bf16, tag="ex", name="ex")
                            nc.scalar.activation(
                                ex[:, :, c0:512],
                                sps[:, :, c0:512],
                                ACTF.Exp,
                                scale=float(D) ** -0.5,
                            )
                            if diag:
                                nc.vector.tensor_tensor(
                                    ex[:, :, c0:c0 + P],
                                    ex[:, :, c0:c0 + P],
                                    tri_sb[:, None, :].to_broadcast([P, 2, P]),
                                    ALU.mult,
                                )
                            exs[jc] = (ex, c0)
                        return run

                    def av_step(jc, hp=hp, av=av, acc_e=acc_e, acc_o=acc_o, exs=exs, ci=ci, njc=njc):
                        def run():
                            ex, c0 = exs.pop(jc)
                            for hi in range(2):
                                nc.tensor.matmul(
                                    av[hi * D:(hi + 1) * D, c0:512],
                                    v_sb[:, jc, hp, hi, :],
                                    ex[:, hi, c0:512],
                                    start=(jc == 0),
                                    stop=(jc == njc - 1),
                                    skip_group_check=True,
                                )
                            acc = acc_e if jc % 2 == 0 else acc_o
                            if jc < 2:
                                # first member of each chain: copy (+ zero the
                                # columns this diag chunk doesn't cover)
                                if c0 > 0:
                                    nc.vector.memset(acc[:, :, 0:c0], 0.0)
                                nc.vector.tensor_copy(
                                    acc[:, :, c0:512], ex[:, :, c0:512]
                                )
                            else:
                                nc.vector.tensor_tensor(
                                    acc[:, :, c0:512],
                                    acc[:, :, c0:512],
                                    ex[:, :, c0:512],
                                    ALU.add,
                                )
                        return run

                    # software-pipelined: S one step ahead of AV
                    cl.append(s_step(0))
                    for jc in range(1, njc):
                        cl.append(s_step(jc))
                        cl.append(av_step(jc - 1))
                    cl.append(av_step(njc - 1))

                    def push_norm(ci=ci, hp=hp, av=av, acc_e=acc_e, acc_o=acc_o):
                        def run():
                            pending.append((ci, hp, av, acc_e, acc_o))
                            if len(pending) > 1:
                                emit_norm(*pending.pop(0))
                        return run

                    cl.append(push_norm())
                if ci >= 1:
                    for tj in range(4 * (ci - 1), 4 * (ci - 1) + 4):
                        cl.append(proj_tile(tj))
                return cl

            # ---------------- fused master schedule ----------------
            for tsl in range(IC + 1):
                qg = qkv_closures(tsl) if tsl < IC else []
                at = attn_closures(tsl - 1) if tsl >= 1 else []
                n, m = len(qg), len(at)
                if n == 0:
                    for c in at:
                        c()
                else:
                    k = 0
                    for i, g in enumerate(qg):
                        g()
                        k2 = (i + 1) * m // n
                        for c in at[k:k2]:
                            c()
                        k = k2
            while pending:
                emit_norm(*pending.pop(0))
            for tj in range(4 * (IC - 1), 4 * (IC - 1) + 4):
                proj_tile(tj)()
    nc.compile()
    return nc


def _get_nc():
    global _NC
    if _NC is None:
        _NC = _build_nc()
    return _NC


def _make_in_maps(x, W_qkv, b_qkv, W_proj):
    import ml_dtypes

    bf = ml_dtypes.bfloat16
    tri = np.ascontiguousarray(np.triu(np.ones((P, P), dtype=np.float32)).astype(bf))
    onesd = np.ones((P, D), dtype=bf)
    in_maps = []
    for c in range(8):
        b, g = divmod(c, 4)
        s = slice(HD * g, HD * g + HD)
        sk = slice(C + HD * g, C + HD * g + HD)
        sv = slice(2 * C + HD * g, 2 * C + HD * g + HD)
        in_maps.append({
            "xT": np.ascontiguousarray(x[b].T.astype(bf)),
            "wq": np.ascontiguousarray(W_qkv[:, s].astype(bf)),
            "wk": np.ascontiguousarray(W_qkv[:, sk].astype(bf)),
            "wv": np.ascontiguousarray(W_qkv[:, sv].astype(bf)),
            "bq": np.ascontiguousarray(b_qkv[s].reshape(2, P).T),
            "bk": np.ascontiguousarray(b_qkv[sk].reshape(2, P).T),
            "bv": np.ascontiguousarray(np.broadcast_to(b_qkv[sv], (P, HD))),
            "wp": np.ascontiguousarray(W_proj[s, :].astype(bf)),
            "tri": tri,
            "onesd": onesd,
        })
    return in_maps


def kernel(x, W_qkv, b_qkv, W_proj, b_proj):
    global LAST_RESULTS
    from concourse import bass_utils

    x = np.asarray(x, dtype=np.float32)
    W_qkv = np.asarray(W_qkv, dtype=np.float32)
    b_qkv = np.asarray(b_qkv, dtype=np.float32)
    W_proj = np.asarray(W_proj, dtype=np.float32)
    b_proj = np.asarray(b_proj, dtype=np.float32)

    nc = _get_nc()
    in_maps = _make_in_maps(x, W_qkv, b_qkv, W_proj)
    res = bass_utils.run_bass_kernel_spmd(nc, in_maps, core_ids=list(range(8)))
    LAST_RESULTS = res
    ys = []
    for b in range(2):
        y = res.results[4 * b]["out"].astype(np.float32)
        for g in range(1, 4):
            y = y + res.results[4 * b + g]["out"].astype(np.float32)
        ys.append(y + b_proj)
    return np.stack(ys, axis=0)
